# revision 36
# baseline (speedup 1.0000x reference)
"""Trainium2 Bass kernel for nn_Attention_46402826666629.

Multi-branch attention with BiasedWedge, Chebyshev phase rotation,
softplus-gated causal attention with learned sink, branch-mean output.

Sharding: the 4 branches share K/V (they are tiled copies), so the 8
cores split as 2 (batch) x 4 (kv-head groups of 3).  Each core handles
all 4 branches for its 3 kv heads: 12 q-heads, 2 K projection pairs
(the 3rd kv head duplicated into both halves of the second pair), a
192-wide V projection, and the output projection rows for its q-heads
scaled by 1/4.  Host sums the 4 partials per batch element.

Layout notes (per core):
  - scores are computed TRANSPOSED: scoresT[s, t] (s = key position on
    partitions, t = query on free dim) so the QK matmul (lhsT = kT,
    rhs = qT), the PV matmul (lhsT = v_aug, rhs = g) and the per-key
    1/sqrt(key_self) scale (per-partition ACT scale) all consume
    natural layouts.
  - softplus(x) = ln(1 + exp(x)) via two ACT passes (exp & ln share no
    table set on this toolchain, so the chain batches same-function
    blocks); the sigmoid gate w*sigmoid(SCALE*w) is silu(SCALE*w)/SCALE
    via one Silu pass; the 1/SCALE is folded into the sink constants.
  - rdn = ATTNSCALE/sqrt(key_self+eps) = exp(-0.5*ln(ks+eps) - ln 8):
    two tiny ACT ops computed directly in key-transposed layout from
    small per-pair matmuls (no sqrt table loads, no PE transposes).
  - the BiasedWedge q + q@S is a single extra matmul accumulated onto
    the projection PSUM with S = (A - A^T)blockdiag + diag(id_bias)
    prepared on the host.
  - the PV accumulator is seeded with outer(snk_aug_row, ones): its
    partition 0 collects SCALE*(rowsum + sink + eps) via the V ones
    column, partitions 64:128 collect the context plus the sink*v_null
    term.  partition_broadcast requires a partition-0 source and DVE
    same-space operands must share a base partition, hence the layout.
  - all inputs are host-packed into a handful of [128, X] tensors so
    each needs ONE dma descriptor (HWDGE descriptor gen is 625ns each
    and serialized; per-tile transfers would gate the pipeline start).
  - emission order pipelines: kt0+Q0 -> group0 exps overlap V
    projection and later Q pairs; the out-projection contribution of
    groups 0/1 is pre-computed during group 2 and re-added in the tail.
"""

import math
from contextlib import ExitStack

import numpy as np
import ml_dtypes

D_MODEL = 768
N_HEAD = 12
N_BR = 4
DH = 64
H_TOT = 48
T = 1024
B = 2
SCALE = math.pi / math.sqrt(3.0)
ATTNSCALE = DH ** -0.5
EPSD = SCALE * 1e-6          # folded eps for the normalization denominator
NT = T // 128                # 8 s-tiles / t-tiles
NCC = D_MODEL // 128         # 6 channel chunks
NKV = 3                      # kv heads per core (4 branches x 3 kv heads)
KTMAP = [0, 0, 0, 0, 1, 1]   # q-pair -> kt tile

# cpackA column offsets: ba | bb2 | psw (rotation consts, loaded early)
CPA_BA = 0
CPA_BB2 = CPA_BA + T
CPA_PSW = CPA_BB2 + T
CPA_W = CPA_PSW + 128
# cpackB column offsets: trimask | wbp (wedge consts, loaded after wq)
CPB_TRI = 0
CPB_WBP = CPB_TRI + 128
CPB_W = CPB_WBP + D_MODEL
# bpack column offsets: bq | bk | bv | bo | snkaug
BPK_BQ = 0
BPK_BK = BPK_BQ + D_MODEL
BPK_BV = BPK_BK + 256
BPK_BO = BPK_BV + NKV * DH
BPK_SNK = BPK_BO + D_MODEL
BPK_W = BPK_SNK + N_HEAD * 128

_CACHE = {}

# dev bisection knobs (defaults = production config)
import os as _os
_MASK = _os.environ.get("KOPT_MASK", "gpsimd")     # gpsimd | dve
_REPEAT = int(_os.environ.get("KOPT_REPEAT", "1"))

UOFF = [0]
for _i in range(1, NT + 1):
    UOFF.append(UOFF[-1] + (T - 128 * (_i - 1)))
ULEN = UOFF[NT]


def _cheb_bases():
    """Replicate reference._chebyshev in exact fp32 arithmetic."""
    f32 = np.float32
    x = (f32(2.0) * np.arange(T, dtype=f32) / f32(T - 1) - f32(1.0)).astype(f32)
    cols = [np.ones_like(x), x]
    maxdeg = max(3, DH)      # 2*H = DH = 64
    for _ in range(2, maxdeg + 1):
        cols.append((f32(2.0) * x * cols[-1] - cols[-2]).astype(f32))
    T_all = np.stack(cols, axis=1)                       # (T, maxdeg+1) fp32
    H = DH // 2
    frac = (np.arange(H, dtype=f32) / f32(H - 1)).astype(f32)
    n_f = np.clip(1 + np.round(frac * f32(maxdeg - 2)).astype(np.int32), 1, maxdeg - 1)
    raw1 = T_all[:, n_f]                                 # (T, H)
    raw2 = T_all[:, n_f + 1]
    nrm = np.sqrt(raw1 * raw1 + raw2 * raw2 + f32(1e-8)).astype(f32)
    b1 = (raw1 / nrm).astype(f32)                        # (T, 32)
    b2 = (raw2 / nrm).astype(f32)
    return b1, b2


def _pack128(a):
    """(NCC*128, W) -> (128, NCC*W) with cc-major columns."""
    ncc = a.shape[0] // 128
    return np.ascontiguousarray(
        a.reshape(ncc, 128, a.shape[1]).transpose(1, 0, 2).reshape(128, -1))


def _emit(ctx: ExitStack, tc, outs, ins, dbg=None):
    import concourse.bass as bass
    from concourse import mybir, library_config

    nc = tc.nc
    F32 = mybir.dt.float32
    BF16 = mybir.dt.bfloat16
    AF = mybir.ActivationFunctionType
    OP = mybir.AluOpType

    y_d = outs["y"]

    def dump(name, ap):
        if dbg is not None and name in dbg:
            nc.sync.dma_start(dbg[name][:], ap)

    # Emission-order chain over ACT ops so the tile scheduler cannot
    # interleave ops from different activation-table sets (table thrash).
    from concourse.tile_rust import add_dep_helper as _adh
    _act_chain = []

    def act(*a, **k):
        bi = nc.scalar.activation(*a, **k)
        if _act_chain:
            _adh(bi.ins, _act_chain[-1].ins, sync=False, reason="act-order")
        _act_chain.append(bi)
        return bi

    nc.gpsimd.load_library(library_config.attn)

    # ---------------- constants (packed DMAs) ----------------
    cpool = ctx.enter_context(tc.tile_pool(name="consts", bufs=1))

    bpk = cpool.tile([1, BPK_W], BF16, tag="bpk", name="bpk")
    nc.sync.dma_start(bpk[:], ins["bpk"][:])
    bq_sb = bpk[:, BPK_BQ:BPK_BQ + D_MODEL]
    bk_sb = bpk[:, BPK_BK:BPK_BK + 256]
    bv_sb = bpk[:, BPK_BV:BPK_BV + NKV * DH]
    bo_sb = bpk[:, BPK_BO:BPK_BO + D_MODEL]
    snkaug = bpk[:, BPK_SNK:BPK_SNK + N_HEAD * 128]

    wz = cpool.tile([128, 640], BF16, tag="wz", name="wz")
    nc.vector.memset(wz[:], 0.001)
    ones16 = cpool.tile([1, T], BF16, tag="ones16", name="ones16")
    nc.vector.memset(ones16[:], 1.0)
    sel2 = cpool.tile([128, 2], BF16, tag="sel2", name="sel2")
    nc.vector.memset(sel2[:], 0.0)
    nc.vector.memset(sel2[0:64, 0:1], 1.0)
    nc.vector.memset(sel2[64:128, 1:2], 1.0)
    eps_c = cpool.tile([128, 1], F32, tag="eps_c", name="eps_c")
    nc.vector.memset(eps_c[:], 1e-6)
    mln8_c = cpool.tile([128, 1], F32, tag="mln8_c", name="mln8_c")
    nc.vector.memset(mln8_c[:], -math.log(8.0))

    # ---------------- persistent big buffers ----------------
    ppool = ctx.enter_context(tc.tile_pool(name="persist", bufs=1))
    qt = [ppool.tile([128, T], BF16, tag=f"qt{p}", name=f"qt{p}") for p in range(6)]
    kt = [ppool.tile([128, T], BF16, tag=f"kt{p}", name=f"kt{p}") for p in range(2)]
    vaug = [ppool.tile([128, NKV * 128], BF16, tag=f"va{i}", name=f"va{i}") for i in range(NT)]
    ctxs = [ppool.tile([128, T], BF16, tag=f"ctx{p}", name=f"ctx{p}") for p in range(6)]
    rdnT = ppool.tile([128, 2 * 2 * NT], F32, tag="rdnT", name="rdnT")  # col ktp*16+2i+jj
    ksq = [ppool.tile([128, 2 * NT], F32, tag=f"ks{p}", name=f"ks{p}") for p in range(2)]
    rkq = [ppool.tile([128, 2 * NT], F32, tag=f"rk{p}", name=f"rk{p}") for p in range(2)]

    for _rep in range(_REPEAT):
      with ExitStack() as ph:
          xpool = ph.enter_context(tc.tile_pool(name="xt", bufs=1))
          wpool = ph.enter_context(tc.tile_pool(name="wld", bufs=1))
          tpool = ph.enter_context(tc.tile_pool(name="ptmp", bufs=1))
          upool = ph.enter_context(tc.tile_pool(name="u", bufs=2))
          nrm_pool = ph.enter_context(tc.tile_pool(name="nrm", bufs=1))
          pps = ph.enter_context(tc.tile_pool(name="pps", bufs=2, space="PSUM"))
          # one 3-deep ring shared by QK score tiles and PV accumulators:
          # 3 x [128,1024]f32 (6 banks) + pps (2) = all 8 PSUM banks
          qkps = ph.enter_context(tc.tile_pool(name="qkps", bufs=3, space="PSUM"))

          wk = wpool.tile([128, NCC * 256], BF16, tag="wk", name="wk")
          nc.sync.dma_start(wk[:], ins["wkt"][:])
          xt_all = xpool.tile([128, NCC * T], BF16, tag="xt", name="xt")
          nc.sync.dma_start(xt_all[:], ins["xt"][:])
          cpa = cpool.tile([128, CPA_W], BF16, tag="cpa", name="cpa")
          nc.sync.dma_start(cpa[:], ins["cpa"][:])
          ba = cpa[:, CPA_BA:CPA_BA + T]
          bb2 = cpa[:, CPA_BB2:CPA_BB2 + T]
          psw = cpa[:, CPA_PSW:CPA_PSW + 128]
          wq = wpool.tile([128, NCC * D_MODEL], BF16, tag="wq", name="wq")
          nc.sync.dma_start(wq[:], ins["wqt"][:])
          cpb = cpool.tile([128, CPB_W], BF16, tag="cpb", name="cpb")
          nc.sync.dma_start(cpb[:], ins["cpb"][:])
          trimask = cpb[:, CPB_TRI:CPB_TRI + 128]
          wbp_sb = cpb[:, CPB_WBP:CPB_WBP + D_MODEL]

          # PE warm-up: contiguous dummy matmuls from program start so the
          # tensor engine is at high p-state when the first projection's
          # weights land (HAM ramps the clock only under continuous
          # execution)
          wps = pps.tile([128, 512], F32, tag="pp", name="pp")
          for _w in range(16):
              nc.tensor.matmul(wps[:], wz[:, 0:128], wz[:, 128:640],
                               start=True, stop=True)

          def proj_pair(w_all, wwidth, bias_sb, boff, dest, p, wedge,
                        act_cp=False):
              """Project one 128-channel pair, rotate, write dest[p].

              For K pairs (wedge=False) also computes key-self sums.
              act_cp routes the PSUM->SBUF copies through the (idle) ACT
              engine -- used for the first K/Q pair to shorten the
              pipeline lead-in."""
              def cp(dst, src_):
                  if act_cp:
                      act(dst, src_, AF.Copy)
                  else:
                      nc.vector.tensor_copy(dst, src_)
              raw = None
              if wedge:
                  raw = tpool.tile([128, T], BF16, tag="raw", name="raw")
              src = tpool.tile([128, T], BF16, tag="src", name="src")
              m1 = tpool.tile([128, T], BF16, tag="m1", name="m1")
              m2 = tpool.tile([128, T], BF16, tag="m2", name="m2")
              for b in range(2):
                  ch0 = b * 512
                  pb = pps.tile([128, 512], F32, tag="pp", name="pp")
                  for cc in range(NCC):
                      nc.tensor.matmul(
                          pb[:],
                          w_all[:, cc * wwidth + p * 128:cc * wwidth + (p + 1) * 128],
                          xt_all[:, cc * T + ch0:cc * T + ch0 + 512],
                          start=(cc == 0), stop=False)
                  nc.tensor.matmul(
                      pb[:], bias_sb[0:1, boff + p * 128:boff + (p + 1) * 128],
                      ones16[0:1, ch0:ch0 + 512], start=False, stop=True)
                  if wedge:
                      cp(raw[:, ch0:ch0 + 512], pb[:])
                      nc.tensor.matmul(
                          pb[:], wbp_sb[:, p * 128:(p + 1) * 128],
                          raw[:, ch0:ch0 + 512], start=False, stop=True,
                          skip_group_check=True)
                  cp(src[:, ch0:ch0 + 512], pb[:])
                  # rotation per bank (shorter latency chain):
                  # dest = src*b1 + psw_signed @ (src*b2)
                  nc.vector.tensor_mul(m1[:, ch0:ch0 + 512],
                                       src[:, ch0:ch0 + 512],
                                       ba[:, ch0:ch0 + 512])
                  nc.vector.tensor_mul(m2[:, ch0:ch0 + 512],
                                       src[:, ch0:ch0 + 512],
                                       bb2[:, ch0:ch0 + 512])
                  sw = pps.tile([128, 512], F32, tag="pp", name="pp")
                  nc.tensor.matmul(sw[:], psw, m2[:, ch0:ch0 + 512],
                                   start=True, stop=True)
                  nc.vector.tensor_add(dest[p][:, ch0:ch0 + 512],
                                       m1[:, ch0:ch0 + 512], sw[:])
              if not wedge:
                  # key_self -> per-pair column sums, transposed layout;
                  # the ACT ln/exp that turn this into rdnT are emitted
                  # separately (rdn_ln/rdn_exp) to keep the ACT chain order
                  # free of stalls.
                  sq = tpool.tile([128, T], BF16, tag="sq", name="sq")
                  nc.vector.tensor_mul(sq[:], dest[p][:], dest[p][:])
                  pk = pps.tile([128, 2 * NT], F32, tag="pp", name="pp")
                  for i in range(NT):
                      nc.tensor.matmul(pk[:, 2 * i:2 * i + 2],
                                       sq[:, i * 128:(i + 1) * 128],
                                       sel2[:], start=True, stop=True)
                  nc.vector.tensor_copy(ksq[p][:], pk[:])

          # rdn = ATTNSCALE/sqrt(ks+eps) = exp(-0.5*ln(ks+eps) - ln 8).
          # Split so each half lands in a same-table-set block of the ACT
          # chain (the table-load pass assigns ln and exp different sets).
          def rdn_ln(p):
              act(rkq[p][:], ksq[p][:], AF.Ln, bias=eps_c[:, 0:1])

          def rdn_exp(p):
              act(rdnT[:, p * 16:p * 16 + 16], rkq[p][:], AF.Exp,
                  scale=-0.5, bias=mln8_c[:, 0:1])

          us_all = {}

          def attn_exps(g, pp_):
              if pp_ == 0:
                  us_all[g] = [upool.tile([128, ULEN], BF16, tag=f"u{j4}",
                                          name=f"u{j4}") for j4 in range(4)]
              us = us_all[g]
              p = 2 * g + pp_
              ktp = KTMAP[p]
              for i in range(NT):
                  v0 = i * 128
                  qks = []
                  for jj in range(2):
                      rows = slice(jj * 64, jj * 64 + 64)
                      qk = qkps.tile([128, T], F32, tag="qk", name="qk")
                      for bank in range(0, T, 512):
                          ch0 = max(v0, bank)
                          chw = bank + 512 - ch0
                          if chw <= 0:
                              continue
                          nc.tensor.matmul(qk[:, ch0:ch0 + chw],
                                           kt[ktp][rows, v0:v0 + 128],
                                           qt[p][rows, ch0:ch0 + chw],
                                           start=True, stop=True)
                      qks.append(qk)
                  for jj in range(2):
                      j4 = 2 * pp_ + jj
                      col = ktp * 16 + 2 * i + jj
                      act(us[j4][:, UOFF[i]:UOFF[i + 1]],
                          qks[jj][:, v0:T], AF.Exp,
                          scale=rdnT[:, col:col + 1])

          def attn_lns(g):
              us = us_all[g]
              for j4 in range(4):
                  act(us[j4][:], us[j4][:], AF.Ln, bias=1.0)
                  # causal mask on diagonal blocks
                  for i in range(NT):
                      dv = us[j4][:, UOFF[i]:UOFF[i] + 128]
                      if _MASK == "gpsimd":
                          nc.gpsimd.affine_select(dv, dv, pattern=[[1, 128]],
                                                  compare_op=OP.is_ge, fill=0.0,
                                                  base=0, channel_multiplier=-1)
                      else:
                          nc.vector.tensor_mul(dv, dv, trimask)

          def attn_finish(g):
              us = us_all.pop(g)
              for j4 in range(4):
                  act(us[j4][:], us[j4][:], AF.Silu, scale=float(SCALE))
              if g == 0 and dbg is not None:
                  dump("du0", us[0][:])
                  dump("du1", us[1][:])
              for j4 in range(4):
                  j = 4 * g + j4
                  p = 2 * g + j4 // 2
                  kv = (j4 % 2) if p < 4 else 2
                  cps = qkps.tile([128, T], F32, tag="qk", name="qk")
                  for b in range(2):
                      b0, b1e = b * 512, (b + 1) * 512
                      nc.tensor.matmul(cps[:, b0:b1e],
                                       snkaug[0:1, j * 128:(j + 1) * 128],
                                       ones16[0:1, b0:b1e],
                                       start=True, stop=False,
                                       skip_group_check=True)
                      imax = min(NT, (b1e - 1) // 128 + 1)
                      for i in range(imax):
                          v0 = i * 128
                          ch0 = max(v0, b0)
                          chw = b1e - ch0
                          nc.tensor.matmul(cps[:, ch0:ch0 + chw],
                                           vaug[i][:, kv * 128:(kv + 1) * 128],
                                           us[j4][:, UOFF[i] + ch0 - v0:
                                                  UOFF[i] + ch0 - v0 + chw],
                                           start=False, stop=(i == imax - 1),
                                           skip_group_check=True)
                  # normalize: ctx = cps[64:128] * (1 / cps[0]); the
                  # denominator row sits at partition 0 (partition_broadcast
                  # needs a partition-0 source), the context at 64:128 (the
                  # final mul mixes PSUM + SBUF operands, exempt from the
                  # equal-base-partition rule)
                  cr = nrm_pool.tile([1, T], F32, tag="cr", name="cr")
                  nc.vector.tensor_copy(cr[:], cps[0:1, :])
                  bx = nrm_pool.tile([64, T], F32, tag="bx", name="bx")
                  nc.gpsimd.partition_broadcast(bx[:], cr[:])
                  rx = nrm_pool.tile([64, T], F32, tag="rx", name="rx")
                  nc.vector.reciprocal_approx_fast(rx[:], bx[:])
                  nc.vector.tensor_mul(
                      ctxs[j // 2][(j % 2) * 64:(j % 2) * 64 + 64, :],
                      cps[DH:2 * DH, :], rx[:])

          # ---- pipeline: kt0+Q0 -> g0 exps; V + later Q pairs overlap ----
          proj_pair(wk, 256, bpk, BPK_BK, kt, 0, wedge=False)
          proj_pair(wq, D_MODEL, bpk, BPK_BQ, qt, 0, wedge=True)
          rdn_ln(0)
          rdn_exp(0)
          attn_exps(0, 0)
          proj_pair(wq, D_MODEL, bpk, BPK_BQ, qt, 1, wedge=True)
          attn_exps(0, 1)
          attn_lns(0)

          proj_pair(wq, D_MODEL, bpk, BPK_BQ, qt, 2, wedge=True)
          proj_pair(wq, D_MODEL, bpk, BPK_BQ, qt, 3, wedge=True)

          # V projection (overlaps group 0 activations on PE/DVE)
          wv = wpool.tile([128, NCC * NKV * DH], BF16, tag="wv", name="wv")
          nc.sync.dma_start(wv[:], ins["wvt"][:])
          vw = NKV * DH
          for i in range(NT):
              dst = vaug[i][:]
              pb = pps.tile([128, 512], F32, tag="pp", name="pp")
              for cc in range(NCC):
                  nc.tensor.matmul(
                      pb[:, 0:vw],
                      xt_all[:, cc * T + i * 128:cc * T + (i + 1) * 128],
                      wv[:, cc * vw:(cc + 1) * vw],
                      start=(cc == 0), stop=False)
              nc.tensor.matmul(
                  pb[:, 0:vw], ones16[0:1, 0:128],
                  bv_sb[0:1, 0:vw], start=False, stop=True)
              dv = bass.AP(dst.tensor, dst.offset + DH,
                           [dst.ap[0], [128, NKV], [1, DH]])
              nc.vector.tensor_copy(
                  dv, pb[:, 0:vw].rearrange("p (h d) -> p h d", h=NKV))
              zc = bass.AP(dst.tensor, dst.offset + 1,
                           [dst.ap[0], [128, NKV], [1, DH - 1]])
              nc.vector.memset(zc, 0.0)
              oc = bass.AP(dst.tensor, dst.offset,
                           [dst.ap[0], [128, NKV], [1, 1]])
              nc.gpsimd.memset(oc, 1.0)

          wo = wpool.tile([128, NCC * D_MODEL], BF16, tag="wo", name="wo")
          nc.sync.dma_start(wo[:], ins["wo"][:])

          attn_finish(0)

          attn_exps(1, 0)
          attn_exps(1, 1)

          proj_pair(wk, 256, bpk, BPK_BK, kt, 1, wedge=False)
          proj_pair(wq, D_MODEL, bpk, BPK_BQ, qt, 4, wedge=True)
          proj_pair(wq, D_MODEL, bpk, BPK_BQ, qt, 5, wedge=True)

          attn_lns(1)
          rdn_ln(1)

          attn_finish(1)

          rdn_exp(1)
          attn_exps(2, 0)
          attn_exps(2, 1)

          # partial out-projection for groups 0/1 heads (ctxs[0..3]) while
          # the ACT engine chews group 2; re-added in the tail
          psb = ppool.tile([128, NT * D_MODEL], BF16, tag="psb", name="psb")
          for tt in range(NT):
              for ch0 in range(0, D_MODEL, 512):
                  chw = min(512, D_MODEL - ch0)
                  pb = pps.tile([128, 512], F32, tag="pp", name="pp")
                  for cc in range(4):
                      nc.tensor.matmul(
                          pb[:, 0:chw],
                          ctxs[cc][:, tt * 128:(tt + 1) * 128],
                          wo[:, cc * D_MODEL + ch0:cc * D_MODEL + ch0 + chw],
                          start=(cc == 0), stop=False)
                  nc.tensor.matmul(
                      pb[:, 0:chw], ones16[0:1, 0:128],
                      bo_sb[0:1, ch0:ch0 + chw], start=False, stop=True)
                  nc.vector.tensor_copy(
                      psb[:, tt * D_MODEL + ch0:tt * D_MODEL + ch0 + chw],
                      pb[:, 0:chw])

          attn_lns(2)
          attn_finish(2)

          for p_ in range(6):
              dump(f"dqt{p_}", qt[p_][:])
          for p_ in range(2):
              dump(f"dkt{p_}", kt[p_][:])
          for i_ in range(NT):
              dump(f"dva{i_}", vaug[i_][:])
          dump("drdnT", rdnT[:])
          for p_ in range(6):
              dump(f"dctx{p_}", ctxs[p_][:])

          # ===== tail: last two ctx chunks + re-add the partial =====
          # (yp accumulators borrow the qk PSUM ring slots)
          ypool = ph.enter_context(tc.tile_pool(name="ysb", bufs=2))
          for tt in range(NT):
              ps = qkps.tile([128, D_MODEL], F32, tag="qk", name="qk")
              for ch0 in range(0, D_MODEL, 512):
                  chw = min(512, D_MODEL - ch0)
                  for cc in range(4, NCC):
                      nc.tensor.matmul(
                          ps[:, ch0:ch0 + chw],
                          ctxs[cc][:, tt * 128:(tt + 1) * 128],
                          wo[:, cc * D_MODEL + ch0:cc * D_MODEL + ch0 + chw],
                          start=(cc == 4), stop=(cc == NCC - 1))
              ysb = ypool.tile([128, D_MODEL], F32, tag="y", name="y")
              nc.vector.tensor_add(ysb[:], ps[:],
                                   psb[:, tt * D_MODEL:(tt + 1) * D_MODEL])
              nc.sync.dma_start(y_d[tt * 128:(tt + 1) * 128, :], ysb[:])


def build(debug=False, repeat=None):
    """Build + compile the 8-core SPMD program (cached)."""
    global _REPEAT
    if repeat is not None:
        _REPEAT = repeat
    key = ("nc", debug, _REPEAT)
    if key in _CACHE:
        return _CACHE[key], _CACHE[("in_aps", debug, _REPEAT)]
    import concourse.tile as tile
    from concourse import bacc, mybir

    F32 = mybir.dt.float32
    BF16 = mybir.dt.bfloat16

    nc = bacc.Bacc("TRN2", target_bir_lowering=False, debug=False,
                   enable_asserts=False, num_devices=8)

    specs = {
        "xt": ((128, NCC * T), BF16),
        "wqt": ((128, NCC * D_MODEL), BF16),
        "wkt": ((128, NCC * 256), BF16),
        "wvt": ((128, NCC * NKV * DH), BF16),
        "wo": ((128, NCC * D_MODEL), BF16),
        "cpa": ((128, CPA_W), BF16),
        "cpb": ((128, CPB_W), BF16),
        "bpk": ((1, BPK_W), BF16),
    }
    in_aps = {k: nc.dram_tensor(k, shape, dt, kind="ExternalInput").ap()
              for k, (shape, dt) in specs.items()}
    out_ap = nc.dram_tensor("y", (T, D_MODEL), F32, kind="ExternalOutput").ap()

    dbg = None
    if debug:
        BF16n = mybir.dt.bfloat16
        dbg = {}
        for p in range(6):
            dbg[f"dqt{p}"] = nc.dram_tensor(f"dqt{p}", (128, T), BF16n, kind="ExternalOutput").ap()
            dbg[f"dctx{p}"] = nc.dram_tensor(f"dctx{p}", (128, T), BF16n, kind="ExternalOutput").ap()
        for p in range(2):
            dbg[f"dkt{p}"] = nc.dram_tensor(f"dkt{p}", (128, T), BF16n, kind="ExternalOutput").ap()
        for i in range(NT):
            dbg[f"dva{i}"] = nc.dram_tensor(f"dva{i}", (128, NKV * 128), BF16n, kind="ExternalOutput").ap()
        dbg["drdnT"] = nc.dram_tensor("drdnT", (128, 2 * 2 * NT), mybir.dt.float32, kind="ExternalOutput").ap()
        dbg["du0"] = nc.dram_tensor("du0", (128, ULEN), BF16n, kind="ExternalOutput").ap()
        dbg["du1"] = nc.dram_tensor("du1", (128, ULEN), BF16n, kind="ExternalOutput").ap()
    with tile.TileContext(nc) as tc:
        with ExitStack() as ctx:
            _emit(ctx, tc, {"y": out_ap}, in_aps, dbg=dbg)
    nc.compile()
    _CACHE[key] = nc
    _CACHE[("in_aps", debug, _REPEAT)] = in_aps
    return nc, in_aps


def make_in_maps(inputs):
    """Shard the full inputs into per-core DRAM maps (layout ops only)."""
    bf16 = ml_dtypes.bfloat16
    f32 = np.float32
    X = np.asarray(inputs["X"], f32)
    Wq = np.asarray(inputs["Wq"], f32)
    bq = np.asarray(inputs["bq"], f32)
    Wk = np.asarray(inputs["Wk"], f32)
    bk = np.asarray(inputs["bk"], f32)
    Wv = np.asarray(inputs["Wv"], f32)
    bv = np.asarray(inputs["bv"], f32)
    A = np.asarray(inputs["A"], f32)
    idb = np.asarray(inputs["id_bias"], f32)
    sink = np.asarray(inputs["sink_scalars"], f32).reshape(H_TOT)
    vn = np.asarray(inputs["v_nulls"], f32).reshape(H_TOT, DH)
    Wo = np.asarray(inputs["Wo"], f32)
    bo = np.asarray(inputs["bo"], f32)

    b1, b2 = _cheb_bases()
    ba = np.concatenate([b1.T] * 4, axis=0)                    # (128, T)
    bb2 = np.concatenate([b2.T] * 4, axis=0)                   # unsigned
    # signed swap: dest[p] += sign(p) * src[swap(p)]
    psw = np.zeros((128, 128), f32)
    for hh in (0, 64):
        for i in range(32):
            psw[hh + 32 + i, hh + i] = -1.0      # first half: -x2
            psw[hh + i, hh + 32 + i] = 1.0       # second half: +x1
    trimask = (np.arange(128)[None, :] >= np.arange(128)[:, None]).astype(f32)

    aat = A - A.T
    wb_base = np.zeros((128, 128), f32)
    wb_base[0:64, 0:64] = aat
    wb_base[64:128, 64:128] = aat

    in_maps = []
    for c in range(8):
        b, q = divmod(c, 4)
        h0, h1, h2 = 3 * q, 3 * q + 1, 3 * q + 2
        # q-head order (pair-major): 4 branch pairs of (h0,h1), then
        # 2 cross-branch pairs of h2: (br0,br1) and (br2,br3)
        heads = [(n, h) for n in range(4) for h in (h0, h1)] + \
                [(0, h2), (1, h2), (2, h2), (3, h2)]
        gidx = [n * N_HEAD + h for n, h in heads]          # global H_TOT index

        wq_sel = np.concatenate([Wq[g * DH:(g + 1) * DH] for g in gidx], axis=0)
        bq_sel = np.concatenate([bq[g * DH:(g + 1) * DH] for g in gidx])
        wk_sel = np.concatenate([Wk[h * DH:(h + 1) * DH]
                                 for h in (h0, h1, h2, h2)], axis=0)
        bk_sel = np.concatenate([bk[h * DH:(h + 1) * DH]
                                 for h in (h0, h1, h2, h2)])
        wv_sel = np.concatenate([Wv[h * DH:(h + 1) * DH]
                                 for h in (h0, h1, h2)], axis=0)
        bv_sel = np.concatenate([bv[h * DH:(h + 1) * DH]
                                 for h in (h0, h1, h2)])
        wo_sel = np.concatenate([Wo[n][h * DH:(h + 1) * DH] for n, h in heads],
                                axis=0)
        bo_sel = f32(0.25) * bo.sum(axis=0) if q == 0 else np.zeros(D_MODEL, f32)

        wbp = np.zeros((128, D_MODEL), f32)
        for p in range(6):
            d0 = idb[gidx[2 * p]]
            d1 = idb[gidx[2 * p + 1]]
            wbp[:, p * 128:(p + 1) * 128] = wb_base + np.diag(
                np.concatenate([d0, d1]))
        sk = sink[gidx]
        snkaug = np.zeros((N_HEAD, 128), f32)
        snkaug[:, 0] = f32(SCALE) * sk + f32(EPSD)
        snkaug[:, DH:] = f32(SCALE) * sk[:, None] * vn[gidx]

        cpa = np.concatenate([ba, bb2, psw], axis=1)
        cpb = np.concatenate([trimask, wbp], axis=1)
        bpk = np.concatenate([bq_sel, bk_sel, bv_sel, bo_sel,
                              snkaug.reshape(-1)])

        in_maps.append({
            "xt": _pack128(X[b].T).astype(bf16),
            "wqt": _pack128(wq_sel.T).astype(bf16),
            "wkt": _pack128(wk_sel.T).astype(bf16),
            "wvt": _pack128(wv_sel.T).astype(bf16),
            "wo": _pack128(wo_sel * f32(0.25)).astype(bf16),
            "cpa": cpa.astype(bf16),
            "cpb": cpb.astype(bf16),
            "bpk": bpk.reshape(1, -1).astype(bf16),
        })
    return in_maps


def run(inputs, trace=False, trace_cores=None):
    from concourse import bass_utils
    nc, _ = build()
    in_maps = make_in_maps(inputs)
    res = bass_utils.run_bass_kernel_spmd(
        nc, in_maps, core_ids=list(range(8)),
        trace=trace, **({"trace_cores": trace_cores} if trace_cores else {}))
    parts = [res.results[c]["y"] for c in range(8)]
    out = np.stack([parts[0] + parts[1] + parts[2] + parts[3],
                    parts[4] + parts[5] + parts[6] + parts[7]], axis=0)
    return out.astype(np.float32), res


def kernel(**inputs):
    out, _ = run(inputs, trace=False)
    return out
